# revision 7
# baseline (speedup 1.0000x reference)
"""AdaptiveKernelFC Trainium2 kernel (8-core data parallel).

Math: the reference builds per-sample filters w[n,p,c,kh,kw] =
x[n,c,kh,kw]*Wk[p] + bk[p] and convolves x[n] with them (7x7 kernel ==
feature map size, pad 3).  The conv factors exactly:

    y[n,p,i,j] = Wk[p]*S1[n,i,j] + bk[p]*S2[n,i,j] + b_adap[p]

with
    S1[n,i,j] = sum_{a,b} G[n,(a,b),(a+i-3,b+j-3)]      (Gram diag bands)
    G[n,r,q]  = sum_c x[n,c,r] * x[n,c,q]               (49x49 per sample)
    S2[n,i,j] = sum_{a,b} xspad[n,i+a,j+b],  xs = sum_c x[n,c]

Pipeline per core (4 samples):
  1. Gram matrices on PE (K=256 contraction), channel sums via ones-matmul.
  2. Copy into zero-padded 13x13 layouts, dump to DRAM scratch.
  3. Diagonal-band gather: per kernel-row `a`, one DMA pulls contiguous
     91-element windows (7x13 padded rows) for all 7 b's x 4 samples into
     an SBUF tile of shape (99, B, 7, 13); rows 0-48 from the Gram, rows
     49-97 from the channel-sum plane, row 98 = ones.
  4. One selector matmul (99,3)^T @ strided (99, B,7,7) view -> [S1;S2;1].
  5. K=3 matmul against [Wk; bk; b_adap] -> all 256 output planes.

Sharding: pure data parallel, batch N=32 split 4 samples/core across 8
cores; params replicated; outputs concatenated.
"""

import os
import numpy as np

import concourse.bass as bass
import concourse.bacc as bacc
import concourse.mybir as mybir
import concourse.tile as tile
from concourse.ap import AP
from concourse.bass_utils import run_bass_kernel_spmd

N, C, H, W = 32, 256, 7, 7
P = 256
NCORES = 8
B = N // NCORES          # samples per core
HW = H * W               # 49
PW = 13                  # padded width (H + 2*3)
PHW = PW * PW            # 169
F32 = mybir.dt.float32

_cached = {}
last_exec_time_ns = None


def build():
    nc = bacc.Bacc(
        "TRN2", target_bir_lowering=False, debug=False, num_devices=NCORES
    )
    x_d = nc.dram_tensor("x", (B, C, H, W), F32, kind="ExternalInput")
    wk_d = nc.dram_tensor("Wk", (P,), F32, kind="ExternalInput")
    bk_d = nc.dram_tensor("bk", (P,), F32, kind="ExternalInput")
    ba_d = nc.dram_tensor("b_adap", (P,), F32, kind="ExternalInput")
    out_d = nc.dram_tensor("out", (B, P, H, W), F32, kind="ExternalOutput")
    # flat scratch with slack: the widened 91-element windows over-read a
    # few elements past the logical end for the bottom-right rows
    gpad_d = nc.dram_tensor("gpad_scratch", (HW * B * PHW + 76,), F32, kind="Internal")
    xspad_d = nc.dram_tensor("xspad_scratch", (B * PHW + 124,), F32, kind="Internal")

    with tile.TileContext(nc) as tc:
        with (
            tc.tile_pool(name="sb", bufs=1) as sb,
            tc.tile_pool(name="ps", bufs=1, space="PSUM") as ps,
        ):
            xsb = sb.tile([128, 2, B, HW], F32)       # x, channel-chunked
            ones = sb.tile([128, 1], F32)
            sel = sb.tile([2 * HW, 2], F32)           # block-row selector
            params = sb.tile([2, P], F32)             # Wk; bk
            badap = sb.tile([128, 2], F32)            # b_adap, chunked
            gpad_sb = sb.tile([HW, B, PW, PW], F32)   # zero-padded Gram
            xspad_sb = sb.tile([1, B, PW, PW], F32)   # zero-padded chan-sums
            EFW = sb.tile([2 * HW, B, 7, PW], F32)    # gathered wide bands
            R = sb.tile([2, B * HW], F32)             # S1; S2
            zeros = sb.tile([1, 128], F32)
            ysb = sb.tile([128, 2, B, HW], F32)

            G_ps = ps.tile([HW, B, HW], F32)
            xs_ps = ps.tile([1, B * HW], F32)
            S_ps = ps.tile([2, B * HW], F32)
            y_ps = ps.tile([128, 2, B, HW], F32)

            sel_np = np.zeros((2 * HW, 2), dtype=np.float32)
            sel_np[0:HW, 0] = 1.0
            sel_np[HW : 2 * HW, 1] = 1.0
            sel_d = nc.inline_tensor(sel_np, name="sel_const")

            nc.vector.memset(ones[:], 1.0)
            nc.vector.memset(zeros[:], 0.0)
            nc.vector.memset(gpad_sb[:], 0.0)
            nc.vector.memset(xspad_sb[:], 0.0)
            nc.sync.dma_start(sel[:], sel_d[:])

            # x -> SBUF with channels on partitions (two 128-chunks)
            xr = x_d.ap().rearrange("n (k c) h w -> k c n (h w)", k=2)
            for ck in range(2):
                nc.sync.dma_start(xsb[:, ck], xr[ck])

            nc.sync.dma_start(params[0:1, :], wk_d.ap().unsqueeze(0))
            nc.sync.dma_start(params[1:2, :], bk_d.ap().unsqueeze(0))
            # b_adap -> (128, 2): partition p, chunk k holds b_adap[k*128+p]
            nc.sync.dma_start(badap[:], AP(ba_d, 0, [[1, 128], [128, 2]]))

            # per-sample spatial Gram matrices, contract over channels
            for b in range(B):
                for ck in range(2):
                    nc.tensor.matmul(
                        G_ps[:, b, :],
                        xsb[:, ck, b, :],
                        xsb[:, ck, b, :],
                        start=(ck == 0),
                        stop=(ck == 1),
                    )
            # channel sums: ones^T @ x
            for ck in range(2):
                nc.tensor.matmul(
                    xs_ps[:, :],
                    ones[:, :],
                    xsb[:, ck].rearrange("c n s -> c (n s)"),
                    start=(ck == 0),
                    stop=(ck == 1),
                )

            # place into zero-padded 13x13 layouts
            nc.scalar.copy(
                gpad_sb[:, :, 3:10, 3:10],
                G_ps[:].rearrange("r b (h w) -> r b h w", h=H),
            )
            nc.scalar.copy(
                xspad_sb[:, :, 3:10, 3:10],
                xs_ps[:].rearrange("o (b h w) -> o b h w", b=B, h=H),
            )

            # bounce through DRAM so the diagonal-band gather can use
            # flat strided access patterns; also zero the over-read slack
            nc.sync.dma_start(AP(gpad_d, 0, [[1, HW * B * PHW]]), gpad_sb[:])
            nc.sync.dma_start(AP(xspad_d, 0, [[1, B * PHW]]), xspad_sb[:])
            nc.sync.dma_start(
                AP(gpad_d, HW * B * PHW, [[1, 76]]), zeros[0:1, 0:76]
            )
            nc.sync.dma_start(
                AP(xspad_d, B * PHW, [[1, 124]]), zeros[0:1, 0:124]
            )

            # widened diagonal-band gather, one DMA per kernel-row a:
            #   EFW[(a,b), n, i, j'] = gpad[(a,b), n, (a*13+b) + i*13 + j']
            # so that EFW[r, n, i, j] = gpad[r, n, a+i, b+j] for j<7
            for a in range(7):
                nc.sync.dma_start(
                    EFW[7 * a : 7 * a + 7],
                    AP(
                        gpad_d,
                        a * (7 * B * PHW + PW),
                        [[B * PHW + 1, 7], [PHW, B], [1, 7 * PW]],
                    ),
                )
                nc.sync.dma_start(
                    EFW[HW + 7 * a : HW + 7 * a + 7],
                    AP(
                        xspad_d,
                        a * PW,
                        [[1, 7], [PHW, B], [1, 7 * PW]],
                    ),
                )

            # block-row reduce on the 7x7 sub-window:
            # S_ps = sel^T @ [E; F; 1] = [S1; S2; 1]
            nc.tensor.matmul(
                S_ps[:],
                sel[:],
                EFW[:, :, :, 0:7],
                start=True,
                stop=True,
            )
            nc.scalar.copy(R[:], S_ps[:])

            # y[p, n, i, j] = Wk[p]*S1 + bk[p]*S2   (+ b_adap via bias)
            for pk in range(2):
                nc.tensor.matmul(
                    y_ps[:, pk],
                    params[:, pk * 128 : (pk + 1) * 128],
                    R[:],
                    start=True,
                    stop=True,
                )
                nc.scalar.activation(
                    ysb[:, pk],
                    y_ps[:, pk],
                    mybir.ActivationFunctionType.Identity,
                    bias=badap[:, pk : pk + 1],
                )
            outr = out_d.ap().rearrange("n (k p) h w -> k p n (h w)", k=2)
            for pk in range(2):
                nc.sync.dma_start(outr[pk], ysb[:, pk])

    nc.compile()
    return nc


def kernel(x, Wk, bk, b_adap):
    global last_exec_time_ns
    if "nc" not in _cached:
        _cached["nc"] = build()
    nc = _cached["nc"]

    x = np.ascontiguousarray(x, dtype=np.float32)
    Wk = np.ascontiguousarray(Wk, dtype=np.float32)
    bk = np.ascontiguousarray(bk, dtype=np.float32)
    b_adap = np.ascontiguousarray(b_adap, dtype=np.float32)

    in_maps = [
        {"x": x[i * B : (i + 1) * B], "Wk": Wk, "bk": bk, "b_adap": b_adap}
        for i in range(NCORES)
    ]
    res = run_bass_kernel_spmd(
        nc,
        in_maps,
        core_ids=list(range(NCORES)),
        trace=bool(os.environ.get("KERNEL_TRACE")),
    )
    last_exec_time_ns = res.exec_time_ns
    out = np.concatenate(
        [res.results[i]["out"].reshape(B, P, H, W) for i in range(NCORES)], axis=0
    )
    return out


# revision 8
# speedup vs baseline: 1.3403x; 1.3403x over previous
"""AdaptiveKernelFC Trainium2 kernel (8-core data parallel).

Math: the reference builds per-sample filters w[n,p,c,kh,kw] =
x[n,c,kh,kw]*Wk[p] + bk[p] and convolves x[n] with them (7x7 kernel ==
feature map size, pad 3).  The conv factors exactly:

    y[n,p,i,j] = Wk[p]*S1[n,i,j] + bk[p]*S2[n,i,j] + b_adap[p]

with
    S1[n,i,j] = sum_{a,b} G[n,(a,b),(a+i-3,b+j-3)]      (Gram diag bands)
    G[n,r,q]  = sum_c x[n,c,r] * x[n,c,q]               (49x49 per sample)
    S2[n,i,j] = sum_{a,b} xspad[n,i+a,j+b],  xs = sum_c x[n,c]

Pipeline per core (4 samples):
  1. One fused matmul pair per sample: lhsT = [x_chunk | ones49] so PSUM
     rows 0-48 are the Gram matrix and rows 49-97 are 49 replicated
     copies of the channel sums.
  2. DVE-copy into a zero-padded 13x13 layout (98, B, 13, 13).
  3. Dump to DRAM with a per-row stagger: row r=(a,b) lands at
     683*r - 13a - b.  This makes the diagonal-band gather uniform:
     element (r, n, i*13+j') sits at 683*r + 169*n + (i*13+j'), so ONE
     3-dim DMA gathers every 91-wide band for all 98 rows x 4 samples.
  4. One selector matmul (98,2)^T @ (98, B,7,7 view) -> [S1; S2].
  5. K=2 matmul against [Wk; bk], then ScalarE Identity-with-bias adds
     b_adap while moving PSUM -> SBUF.

Sharding: pure data parallel, batch N=32 split 4 samples/core across 8
cores; params replicated; outputs concatenated.
"""

import os
import numpy as np

import concourse.bass as bass
import concourse.bacc as bacc
import concourse.mybir as mybir
import concourse.tile as tile
from concourse.ap import AP
from concourse.bass_utils import run_bass_kernel_spmd

N, C, H, W = 32, 256, 7, 7
P = 256
NCORES = 8
B = N // NCORES          # samples per core
HW = H * W               # 49
PW = 13                  # padded width (H + 2*3)
PHW = PW * PW            # 169
ROWSZ = B * PHW          # 676 payload per dumped row
RSTRIDE = ROWSZ + 7      # 683 staggered row stride
F32 = mybir.dt.float32

_cached = {}
last_exec_time_ns = None


def build():
    nc = bacc.Bacc(
        "TRN2", target_bir_lowering=False, debug=False, num_devices=NCORES
    )
    x_d = nc.dram_tensor("x", (B, C, H, W), F32, kind="ExternalInput")
    wk_d = nc.dram_tensor("Wk", (P,), F32, kind="ExternalInput")
    bk_d = nc.dram_tensor("bk", (P,), F32, kind="ExternalInput")
    ba_d = nc.dram_tensor("b_adap", (P,), F32, kind="ExternalInput")
    out_d = nc.dram_tensor("out", (B, P, H, W), F32, kind="ExternalOutput")
    stag_d = nc.dram_tensor(
        "stag_scratch", (2 * HW * RSTRIDE + 800,), F32, kind="Internal"
    )

    with tile.TileContext(nc) as tc:
        with (
            tc.tile_pool(name="sb", bufs=1) as sb,
            tc.tile_pool(name="ps", bufs=1, space="PSUM") as ps,
        ):
            # x columns 0:49, ones columns 49:98 (Gram + chan-sum fused)
            xsb = sb.tile([128, 2, B, 2 * HW], F32)
            sel = sb.tile([2 * HW, 2], F32)           # block-row selector
            params = sb.tile([2, P], F32)             # Wk; bk
            badap = sb.tile([128, 2], F32)            # b_adap, chunked
            gpad_sb = sb.tile([2 * HW, B, PW, PW], F32)
            EFW = sb.tile([2 * HW, B, 7, PW], F32)    # gathered wide bands
            R = sb.tile([2, B * HW], F32)             # S1; S2
            ysb = sb.tile([128, 2, B, HW], F32)

            GX_ps = ps.tile([2 * HW, B, HW], F32)
            S_ps = ps.tile([2, B * HW], F32)
            y_ps = ps.tile([128, 2, B, HW], F32)

            sel_np = np.zeros((2 * HW, 2), dtype=np.float32)
            sel_np[0:HW, 0] = 1.0
            sel_np[HW : 2 * HW, 1] = 1.0
            sel_d = nc.inline_tensor(sel_np, name="sel_const")

            nc.vector.memset(xsb[:, :, :, HW : 2 * HW], 1.0)
            nc.vector.memset(gpad_sb[:], 0.0)
            nc.gpsimd.dma_start(sel[:], sel_d[:])

            # x -> SBUF with channels on partitions (two 128-chunks)
            xr = x_d.ap().rearrange("n (k c) h w -> k c n (h w)", k=2)
            nc.sync.dma_start(xsb[:, 0, :, 0:HW], xr[0])
            nc.scalar.dma_start(xsb[:, 1, :, 0:HW], xr[1])

            nc.gpsimd.dma_start(params[0:1, :], wk_d.ap().unsqueeze(0))
            nc.gpsimd.dma_start(params[1:2, :], bk_d.ap().unsqueeze(0))
            # b_adap -> (128, 2): partition p, chunk k holds b_adap[k*128+p]
            nc.gpsimd.dma_start(badap[:], AP(ba_d, 0, [[1, 128], [128, 2]]))

            # fused Gram + replicated channel-sum rows, contract channels
            for b in range(B):
                for ck in range(2):
                    nc.tensor.matmul(
                        GX_ps[:, b, :],
                        xsb[:, ck, b, :],
                        xsb[:, ck, b, 0:HW],
                        start=(ck == 0),
                        stop=(ck == 1),
                    )

            # place into zero-padded 13x13 layout
            nc.vector.tensor_copy(
                gpad_sb[:, :, 3:10, 3:10],
                GX_ps[:].rearrange("r b (h w) -> r b h w", h=H),
            )

            # staggered dump: row r=(a,b) of block blk at
            #   blk*49*683 + 683*r - 13*a - b
            for blk in range(2):
                nc.sync.dma_start(
                    AP(
                        stag_d,
                        blk * HW * RSTRIDE,
                        [[7 * RSTRIDE - PW, 7], [RSTRIDE - 1, 7], [1, ROWSZ]],
                    ),
                    gpad_sb[blk * HW : (blk + 1) * HW],
                )

            # ONE uniform gather: EFW[r, n, i, j'] = stag[683r + 169n + 13i + j']
            nc.sync.dma_start(
                EFW[:],
                AP(stag_d, 0, [[RSTRIDE, 2 * HW], [PHW, B], [1, 7 * PW]]),
            )

            # block-row reduce on the 7x7 sub-window: S_ps = [S1; S2]
            nc.tensor.matmul(
                S_ps[:],
                sel[:],
                EFW[:, :, :, 0:7],
                start=True,
                stop=True,
            )
            nc.vector.tensor_copy(R[:], S_ps[:])

            # y[p, n, i, j] = Wk[p]*S1 + bk[p]*S2   (+ b_adap via bias)
            outr = out_d.ap().rearrange("n (k p) h w -> k p n (h w)", k=2)
            for pk in range(2):
                nc.tensor.matmul(
                    y_ps[:, pk],
                    params[:, pk * 128 : (pk + 1) * 128],
                    R[:],
                    start=True,
                    stop=True,
                )
                nc.scalar.activation(
                    ysb[:, pk],
                    y_ps[:, pk],
                    mybir.ActivationFunctionType.Identity,
                    bias=badap[:, pk : pk + 1],
                )
                (nc.sync if pk == 0 else nc.scalar).dma_start(outr[pk], ysb[:, pk])

    nc.compile()
    return nc


def kernel(x, Wk, bk, b_adap):
    global last_exec_time_ns
    if "nc" not in _cached:
        _cached["nc"] = build()
    nc = _cached["nc"]

    x = np.ascontiguousarray(x, dtype=np.float32)
    Wk = np.ascontiguousarray(Wk, dtype=np.float32)
    bk = np.ascontiguousarray(bk, dtype=np.float32)
    b_adap = np.ascontiguousarray(b_adap, dtype=np.float32)

    in_maps = [
        {"x": x[i * B : (i + 1) * B], "Wk": Wk, "bk": bk, "b_adap": b_adap}
        for i in range(NCORES)
    ]
    res = run_bass_kernel_spmd(
        nc,
        in_maps,
        core_ids=list(range(NCORES)),
        trace=bool(os.environ.get("KERNEL_TRACE")),
    )
    last_exec_time_ns = res.exec_time_ns
    out = np.concatenate(
        [res.results[i]["out"].reshape(B, P, H, W) for i in range(NCORES)], axis=0
    )
    return out


# revision 9
# speedup vs baseline: 1.4684x; 1.0955x over previous
"""AdaptiveKernelFC Trainium2 kernel (8-core data parallel).

Math: the reference builds per-sample filters w[n,p,c,kh,kw] =
x[n,c,kh,kw]*Wk[p] + bk[p] and convolves x[n] with them (7x7 kernel ==
feature map size, pad 3).  The conv factors exactly:

    y[n,p,i,j] = Wk[p]*S1[n,i,j] + bk[p]*S2[n,i,j] + b_adap[p]

with
    S1[n,i,j] = sum_{a,b} G[n,(a,b),(a+i-3,b+j-3)]      (Gram diag bands)
    G[n,r,q]  = sum_c x[n,c,r] * x[n,c,q]               (49x49 per sample)
    S2[n,i,j] = sum_{a,b} xspad[n,i+a,j+b],  xs = sum_c x[n,c]

Pipeline per core (4 samples):
  1. One fused matmul pair per sample: lhsT = [x_chunk | ones49] so PSUM
     rows 0-48 are the Gram matrix and rows 49-97 are 49 replicated
     copies of the channel sums.
  2. DVE-copy into a zero-padded 13x13 layout (98, B, 13, 13).
  3. Dump to DRAM with a per-row stagger: row r=(a,b) lands at
     683*r - 13a - b.  This makes the diagonal-band gather uniform:
     element (r, n, i*13+j') sits at 683*r + 169*n + (i*13+j'), so ONE
     3-dim DMA gathers every 91-wide band for all 98 rows x 4 samples.
  4. One selector matmul (98,2)^T @ (98, B,7,7 view) -> [S1; S2].
  5. K=2 matmul against [Wk; bk], then ScalarE Identity-with-bias adds
     b_adap while moving PSUM -> SBUF.

Sharding: pure data parallel, batch N=32 split 4 samples/core across 8
cores; params replicated; outputs concatenated.
"""

import os
import numpy as np

import concourse.bass as bass
import concourse.bacc as bacc
import concourse.mybir as mybir
import concourse.tile as tile
from concourse.ap import AP
from concourse.bass_utils import run_bass_kernel_spmd

N, C, H, W = 32, 256, 7, 7
P = 256
NCORES = 8
B = N // NCORES          # samples per core
HW = H * W               # 49
PW = 13                  # padded width (H + 2*3)
PHW = PW * PW            # 169
ROWSZ = B * PHW          # 676 payload per dumped row
RSTRIDE = ROWSZ + 7      # 683 staggered row stride
F32 = mybir.dt.float32

_cached = {}
last_exec_time_ns = None


def build():
    nc = bacc.Bacc(
        "TRN2", target_bir_lowering=False, debug=False, num_devices=NCORES
    )
    x_d = nc.dram_tensor("x", (B, C, H, W), F32, kind="ExternalInput")
    wk_d = nc.dram_tensor("Wk", (P,), F32, kind="ExternalInput")
    bk_d = nc.dram_tensor("bk", (P,), F32, kind="ExternalInput")
    ba_d = nc.dram_tensor("b_adap", (P,), F32, kind="ExternalInput")
    out_d = nc.dram_tensor("out", (B, P, H, W), F32, kind="ExternalOutput")
    stagE_d = nc.dram_tensor(
        "stagE_scratch", (HW * RSTRIDE + 800,), F32, kind="Internal"
    )
    stagF_d = nc.dram_tensor(
        "stagF_scratch", (HW * RSTRIDE + 800,), F32, kind="Internal"
    )

    with tile.TileContext(nc) as tc:
        with (
            tc.tile_pool(name="sb", bufs=1) as sb,
            tc.tile_pool(name="ps", bufs=1, space="PSUM") as ps,
        ):
            # x columns 0:49, ones columns 49:98 (Gram + chan-sum fused)
            xsb = sb.tile([128, 2, B, 2 * HW], F32)
            sel = sb.tile([2 * HW, 2], F32)           # block-row selector
            params = sb.tile([2, P], F32)             # Wk; bk
            badap = sb.tile([128, 2], F32)            # b_adap, chunked
            gpad_sb = sb.tile([2 * HW, B, PW, PW], F32)
            EFW = sb.tile([2 * HW, B, 7, PW], F32)    # gathered wide bands
            R = sb.tile([2, B * HW], F32)             # S1; S2
            ysb = sb.tile([128, 2, B, HW], F32)

            GX_ps = ps.tile([2 * HW, B, HW], F32)
            S_ps = ps.tile([2, B * HW], F32)
            y0_ps = ps.tile([128, B * HW], F32)
            y1_ps = ps.tile([128, B * HW], F32)

            sel_np = np.zeros((2 * HW, 2), dtype=np.float32)
            sel_np[0:HW, 0] = 1.0
            sel_np[HW : 2 * HW, 1] = 1.0
            sel_d = nc.inline_tensor(sel_np, name="sel_const")

            nc.vector.memset(xsb[:, :, :, HW : 2 * HW], 1.0)
            nc.vector.memset(gpad_sb[:], 0.0)
            nc.gpsimd.dma_start(sel[:], sel_d[:])

            # x -> SBUF with channels on partitions (two 128-chunks),
            # split 4 ways for DMA queue parallelism
            xr = x_d.ap().rearrange("n (k c) h w -> k c n (h w)", k=2)
            nc.sync.dma_start(xsb[:, 0, 0:2, 0:HW], xr[0, :, 0:2])
            nc.scalar.dma_start(xsb[:, 0, 2:4, 0:HW], xr[0, :, 2:4])
            nc.sync.dma_start(xsb[:, 1, 0:2, 0:HW], xr[1, :, 0:2])
            nc.scalar.dma_start(xsb[:, 1, 2:4, 0:HW], xr[1, :, 2:4])

            nc.gpsimd.dma_start(params[0:1, :], wk_d.ap().unsqueeze(0))
            nc.gpsimd.dma_start(params[1:2, :], bk_d.ap().unsqueeze(0))
            # b_adap -> (128, 2): partition p, chunk k holds b_adap[k*128+p]
            nc.gpsimd.dma_start(badap[:], AP(ba_d, 0, [[1, 128], [128, 2]]))

            # fused Gram + replicated channel-sum rows, contract channels
            for b in range(B):
                for ck in range(2):
                    nc.tensor.matmul(
                        GX_ps[:, b, :],
                        xsb[:, ck, b, :],
                        xsb[:, ck, b, 0:HW],
                        start=(ck == 0),
                        stop=(ck == 1),
                    )

            # place into zero-padded 13x13 layout
            nc.vector.tensor_copy(
                gpad_sb[:, :, 3:10, 3:10],
                GX_ps[:].rearrange("r b (h w) -> r b h w", h=H),
            )

            # staggered dump: row r=(a,b) lands at 683*r - 13*a - b, so
            # the band gather below has a uniform row stride.  E rows on
            # the Sync DGE, F rows on the Activation DGE - two independent
            # dump->gather chains that pipeline.
            stag_pat = [[7 * RSTRIDE - PW, 7], [RSTRIDE - 1, 7], [1, ROWSZ]]
            gat_pat = [[RSTRIDE, HW], [PHW, B], [1, 7 * PW]]
            nc.sync.dma_start(AP(stagE_d, 0, stag_pat), gpad_sb[0:HW])
            nc.scalar.dma_start(AP(stagF_d, 0, stag_pat), gpad_sb[HW : 2 * HW])
            nc.sync.dma_start(EFW[0:HW], AP(stagE_d, 0, gat_pat))
            nc.scalar.dma_start(EFW[HW : 2 * HW], AP(stagF_d, 0, gat_pat))

            # block-row reduce on the 7x7 sub-window: S_ps = [S1; S2]
            nc.tensor.matmul(
                S_ps[:],
                sel[:],
                EFW[:, :, :, 0:7],
                start=True,
                stop=True,
            )
            nc.vector.tensor_copy(R[:], S_ps[:])

            # y[p, n, i, j] = Wk[p]*S1 + bk[p]*S2   (+ b_adap via bias)
            outr = out_d.ap().rearrange("n (k p) h w -> k p n (h w)", k=2)
            for pk, yps in enumerate([y0_ps, y1_ps]):
                nc.tensor.matmul(
                    yps[:],
                    params[:, pk * 128 : (pk + 1) * 128],
                    R[:],
                    start=True,
                    stop=True,
                )
                nc.scalar.activation(
                    ysb[:, pk],
                    yps[:].rearrange("p (b s) -> p b s", b=B),
                    mybir.ActivationFunctionType.Identity,
                    bias=badap[:, pk : pk + 1],
                )
                (nc.sync if pk == 0 else nc.scalar).dma_start(outr[pk], ysb[:, pk])

    nc.compile()
    return nc


def kernel(x, Wk, bk, b_adap):
    global last_exec_time_ns
    if "nc" not in _cached:
        _cached["nc"] = build()
    nc = _cached["nc"]

    x = np.ascontiguousarray(x, dtype=np.float32)
    Wk = np.ascontiguousarray(Wk, dtype=np.float32)
    bk = np.ascontiguousarray(bk, dtype=np.float32)
    b_adap = np.ascontiguousarray(b_adap, dtype=np.float32)

    in_maps = [
        {"x": x[i * B : (i + 1) * B], "Wk": Wk, "bk": bk, "b_adap": b_adap}
        for i in range(NCORES)
    ]
    res = run_bass_kernel_spmd(
        nc,
        in_maps,
        core_ids=list(range(NCORES)),
        trace=bool(os.environ.get("KERNEL_TRACE")),
    )
    last_exec_time_ns = res.exec_time_ns
    out = np.concatenate(
        [res.results[i]["out"].reshape(B, P, H, W) for i in range(NCORES)], axis=0
    )
    return out
